# revision 8
# baseline (speedup 1.0000x reference)
"""MultiHeadGAT layer on 8 trn2 NeuronCores, data-parallel over batch.

Per core (one batch element), with softmax-invariant rescaling: dividing the
unnormalized attention P[j,i] = exp(leaky_relu(e_src[i]+e_dst[j])) by
exp(e_src[i]) (a per-i factor that cancels in the softmax) gives

  P'[j,i] = max( exp(-0.8*e_src[i]) * exp(0.2*e_dst[j]),  exp(e_dst[j]) )

i.e. ONE fused DVE tensor_scalar op per [128,1024] tile (mult + max against
two per-partition scalars) in bf16 -- no exps in the main loop at all.
Mask multiply runs as one [128,2048] bf16 tensor_tensor per HEAD PAIR
(adjacency duplicated along the free dim), all on DVE (gpsimd contends for
SBUF ports and slows DVE ~4x -- measured).  AV matmul in bf16 (1 cycle/row)
with a ones column appended to the lhsT so row 64 of the accumulator is the
softmax denominator.  Output staged per row-block and stored in 8 batched
DMAs.

Host-side prep (layout/dtype only): h.T, adj.T as bf16, W and W@A as bf16.
"""
import sys

sys.path.insert(0, "/opt/trn_rl_repo")

import numpy as np
import ml_dtypes

import concourse.bass as bass
import concourse.mybir as mybir
import concourse.tile as tile
from concourse.bass_utils import run_bass_kernel_spmd
from concourse.masks import make_identity

F32 = mybir.dt.float32
BF16 = mybir.dt.bfloat16
AF = mybir.ActivationFunctionType
ALU = mybir.AluOpType
BF16NP = ml_dtypes.bfloat16

N_CORES = 8
N = 1024
NB = 8          # row blocks of 128
FIN = 256
KT = 2          # FIN / 128
FO = 512        # heads * fo
H = 8
FOH = 64
ALPHA = 0.2

NSEL = 8        # all heads broadcast via PE selector matmuls

_MAX_SYNC_WAITS = 1


def _split_sync_waits(nc, max_waits=_MAX_SYNC_WAITS):
    """This walrus build rejects instructions carrying more than one sync
    wait; hoist extras onto NOPs inserted just before, on the same engine."""
    uid = 0
    for f in nc.m.functions:
        for bb in f.blocks:
            out = []
            for inst in bb.instructions:
                si = getattr(inst, "sync_info", None)
                if si is not None and si.on_wait and len(si.on_wait) > max_waits:
                    waits = list(si.on_wait)
                    keep = waits[-max_waits:]
                    extra = waits[:-max_waits]
                    si.on_wait.clear()
                    si.on_wait.extend(keep)
                    while extra:
                        chunk, extra = extra[:max_waits], extra[max_waits:]
                        nop = mybir.InstNoOp(
                            name=f"waitsplit-{uid}",
                            engine=inst.engine,
                            sync_info=mybir.SyncInfo(
                                on_wait=list(chunk), on_update=[]
                            ),
                            bass_nofuse=True,
                        )
                        uid += 1
                        out.append(nop)
                out.append(inst)
            bb.instructions[:] = out


def build_nc(split=True):
    nc = bass.Bass()
    hT_d = nc.declare_dram_parameter("hTb", [FIN, N], BF16, isOutput=False)
    adjT_d = nc.declare_dram_parameter("adjT2", [N, 2 * N], BF16, isOutput=False)
    w_d = nc.declare_dram_parameter("Wb", [FIN, FO], BF16, isOutput=False)
    wa_d = nc.declare_dram_parameter("WAb", [FIN, 2 * H], BF16, isOutput=False)
    out_d = nc.declare_dram_parameter("out", [N, FO], F32, isOutput=True)

    with tile.TileContext(nc) as tc:
        with (
            tc.tile_pool(name="const", bufs=1) as const,
            tc.tile_pool(name="persist", bufs=1) as persist,
            tc.tile_pool(name="tp8", bufs=5) as tpool,
            tc.tile_pool(name="epi", bufs=3) as epi,
            tc.tile_pool(name="psS", bufs=3, space="PSUM") as psS,
            tc.tile_pool(name="psAcc", bufs=1, space="PSUM") as psAcc,
        ):
            ident = const.tile([128, 128], F32, tag="ident")
            make_identity(nc, ident[:])

            wa = []
            for k in range(KT):
                t = const.tile([128, 2 * H], BF16, tag=f"WA{k}", name=f"WA{k}")
                nc.sync.dma_start(t[:], wa_d[k * 128:(k + 1) * 128, :])
                wa.append(t)
            hT = [persist.tile([128, N], BF16, tag=f"hT{k}", name=f"hT{k}")
                  for k in range(KT)]
            for k in range(KT):
                nc.sync.dma_start(hT[k][:], hT_d[k * 128:(k + 1) * 128, :])
            wk = []
            for k in range(KT):
                t = const.tile([128, FO], BF16, tag=f"W{k}", name=f"W{k}")
                nc.sync.dma_start(t[:], w_d[k * 128:(k + 1) * 128, :])
                wk.append(t)
            # adjacency duplicated along free dim (host-prepped): one TT
            # masks a head pair
            adjT2 = [persist.tile([128, 2 * N], BF16, tag=f"adjT{j}",
                                  name=f"adjT{j}")
                     for j in range(NB)]
            for jb in range(NB):
                nc.sync.dma_start(
                    adjT2[jb][:], adjT_d[jb * 128:(jb + 1) * 128, :]
                )

            # ---- E_T[16, i] = (WA.T @ hT): rows 0..7 e_src, 8..15 e_dst ----
            e_t = const.tile([16, N], F32, tag="eT")
            for c in range(2):
                ps = psS.tile([16, 512], F32, tag="ps")
                for k in range(KT):
                    nc.tensor.matmul(
                        ps[:], wa[k][:], hT[k][:, c * 512:(c + 1) * 512],
                        start=(k == 0), stop=(k == KT - 1),
                    )
                nc.vector.tensor_copy(e_t[:, c * 512:(c + 1) * 512], ps[:])

            # ---- G8[h, i] = exp(-(1-alpha) * e_src[h, i]) as bf16 ----
            g8 = const.tile([8, N], BF16, tag="g8")
            nc.scalar.activation(g8[:], e_t[0:8, :], AF.Exp, scale=-(1.0 - ALPHA))

            # ---- e_sb[jb][p, 16] = E_T[:, jb*128+p]; s0/s1 = per-j scalars ----
            e_sb = [persist.tile([128, 16], F32, tag=f"E{j}", name=f"E{j}")
                    for j in range(NB)]
            s0sb = [persist.tile([128, H], F32, tag=f"s0{j}", name=f"s0{j}")
                    for j in range(NB)]
            s1sb = [persist.tile([128, H], F32, tag=f"s1{j}", name=f"s1{j}")
                    for j in range(NB)]
            for jb in range(NB):
                tp = psS.tile([128, 512], F32, tag="ps")
                nc.tensor.transpose(
                    tp[:, 0:16], e_t[:, jb * 128:(jb + 1) * 128],
                    ident[0:16, 0:16],
                )
                nc.vector.tensor_copy(e_sb[jb][:], tp[:, 0:16])

            # ---- Gb broadcast over partitions via PE selector matmuls.
            # Emission order feeds pair 0 first: heads 0-1, then jb=0 s-cols,
            # then the rest -- PE and ACT are otherwise idle here. ----
            gbsel = [persist.tile([128, N], BF16, tag=f"gb{hh}", name=f"gb{hh}")
                     for hh in range(H)]
            sel = []
            for hh in range(H):
                t = const.tile([8, 128], BF16, tag=f"sel{hh}", name=f"sel{hh}")
                nc.gpsimd.memset(t[:], 0.0)
                nc.gpsimd.affine_select(
                    out=t[:], in_=t[:], pattern=[[0, 128]],
                    compare_op=ALU.not_equal, fill=1.0,
                    base=-hh, channel_multiplier=1,
                )
                sel.append(t)

            def bcast_head(hh):
                for c in range(2):
                    ps = psS.tile([128, 512], F32, tag="ps")
                    nc.tensor.matmul(
                        ps[:], sel[hh][:], g8[:, c * 512:(c + 1) * 512],
                        start=True, stop=True,
                    )
                    nc.scalar.copy(
                        gbsel[hh][:, c * 512:(c + 1) * 512], ps[:]
                    )

            def scols(jb):
                # s0 = exp(alpha * e_dst), s1 = exp(e_dst)
                nc.scalar.activation(
                    s0sb[jb][:], e_sb[jb][:, 8:16], AF.Exp, scale=ALPHA,
                )
                nc.scalar.activation(
                    s1sb[jb][:], e_sb[jb][:, 8:16], AF.Exp, scale=1.0,
                )

            for hh in range(2):
                bcast_head(hh)
            for jb in range(NB):
                scols(jb)
            for hh in range(2, H):
                bcast_head(hh)

            def gb(hh):
                return gbsel[hh][:, :]

            # ---- wh_aug[jb][j, h, 0:64] = (h @ W) block bf16, [:, h, 64] = 1 ----
            wh_aug = [persist.tile([128, H, 65], BF16, tag=f"wha{j}",
                                   name=f"wha{j}")
                      for j in range(NB)]
            for jb in range(NB):
                ps = psS.tile([128, H, FOH], F32, tag="ps")
                for k in range(KT):
                    nc.tensor.matmul(
                        ps[:, :, :], hT[k][:, jb * 128:(jb + 1) * 128], wk[k][:],
                        start=(k == 0), stop=(k == KT - 1),
                    )
                nc.scalar.activation(
                    wh_aug[jb][:, :, 0:64], ps[:, :, :], AF.Copy,
                )
                nc.gpsimd.memset(wh_aug[jb][:, :, 64:65], 1.0)

            # ---- output staging: osm_all[cb][p, h*64+f] ----
            osm_all = [persist.tile([128, FO], F32, tag=f"osm{c}",
                                    name=f"osm{c}")
                       for c in range(NB)]

            # ---- main attention loop, head pairs ----
            for hp in range(H // 2):
                h0, h1 = 2 * hp, 2 * hp + 1
                acc = {
                    (hh, c): psAcc.tile([65, 512], F32, tag=f"acc{hh % 2}{c}",
                                        name=f"acc{hh % 2}{c}")
                    for hh in (h0, h1) for c in range(2)
                }
                for jb in range(NB):
                    t2 = tpool.tile([128, 2 * N], BF16, tag="t2")
                    for q, hh in enumerate((h0, h1)):
                        nc.vector.tensor_scalar(
                            t2[:, q * N:(q + 1) * N], gb(hh),
                            s0sb[jb][:, hh:hh + 1], s1sb[jb][:, hh:hh + 1],
                            ALU.mult, ALU.max,
                        )
                    nc.vector.tensor_mul(t2[:], t2[:], adjT2[jb][:])
                    for q, hh in enumerate((h0, h1)):
                        for c in range(2):
                            nc.tensor.matmul(
                                acc[(hh, c)][:],
                                wh_aug[jb][:, hh, :],
                                t2[:, q * N + c * 512:q * N + (c + 1) * 512],
                                start=(jb == 0), stop=(jb == NB - 1),
                            )
                # epilogue: acc -> SBUF (ACT), transpose back, divide, stage
                for hh in (h0, h1):
                    acc_sb = epi.tile([65, N], F32, tag="accsb")
                    rec8 = epi.tile([128, 8], F32, tag="rec8")
                    for q in range(2):
                        nc.scalar.copy(
                            acc_sb[:, q * 512:(q + 1) * 512], acc[(hh, q)][:]
                        )
                        tp = psS.tile([128, 4 * 65], F32, tag="ps")
                        for r in range(4):
                            cb = q * 4 + r
                            nc.tensor.transpose(
                                tp[:, r * 65:r * 65 + 65],
                                acc_sb[:, cb * 128:(cb + 1) * 128],
                                ident[0:65, 0:65],
                            )
                        nc.vector.reciprocal(
                            rec8[:, q * 4:(q + 1) * 4], tp[:, 64::65]
                        )
                        for r in range(4):
                            cb = q * 4 + r
                            nc.scalar.activation(
                                osm_all[cb][:, hh * FOH:(hh + 1) * FOH],
                                tp[:, r * 65:r * 65 + 64], AF.Copy,
                                scale=rec8[:, cb:cb + 1],
                            )
                # flush this pair's 128-col quarter of the output
                for cb in range(NB):
                    nc.sync.dma_start(
                        out_d[cb * 128:(cb + 1) * 128,
                              hp * 128:(hp + 1) * 128],
                        osm_all[cb][:, hp * 128:(hp + 1) * 128],
                    )

    if split:
        _split_sync_waits(nc)
    return nc


_NC_CACHE = None


def _get_nc():
    global _NC_CACHE
    if _NC_CACHE is None:
        _NC_CACHE = build_nc()
    return _NC_CACHE


def _dup_adjT(adj_c):
    at = np.ascontiguousarray(adj_c.T).astype(BF16NP)
    return np.ascontiguousarray(np.concatenate([at, at], axis=1))


def _prep_in_maps(h, adj, W, a):
    h = np.ascontiguousarray(h, dtype=np.float32)
    adj = np.ascontiguousarray(adj, dtype=np.int32)
    W = np.ascontiguousarray(W, dtype=np.float32)
    a = np.ascontiguousarray(a, dtype=np.float32)
    amat = np.zeros((FO, 2 * H), dtype=np.float32)
    for hh in range(H):
        amat[hh * FOH:(hh + 1) * FOH, hh] = a[hh, :FOH]
        amat[hh * FOH:(hh + 1) * FOH, H + hh] = a[hh, FOH:]
    wamat = (W @ amat).astype(BF16NP)
    wb = W.astype(BF16NP)
    return [
        {
            "hTb": np.ascontiguousarray(h[c].T).astype(BF16NP),
            "adjT2": _dup_adjT(adj[c]),
            "Wb": wb,
            "WAb": wamat,
        }
        for c in range(N_CORES)
    ]


def run(h, adj, W, a, trace=False, **kw):
    nc = _get_nc()
    in_maps = _prep_in_maps(h, adj, W, a)
    res = run_bass_kernel_spmd(nc, in_maps, list(range(N_CORES)), trace=trace, **kw)
    out = np.stack([res.results[c]["out"] for c in range(N_CORES)], axis=0)
    return out.astype(np.float32), res


def kernel(h, adj, W, a):
    out, _ = run(h, adj, W, a)
    return out


# revision 9
# speedup vs baseline: 1.0221x; 1.0221x over previous
"""MultiHeadGAT layer on 8 trn2 NeuronCores, data-parallel over batch.

Per core (one batch element), with softmax-invariant rescaling: dividing the
unnormalized attention P[j,i] = exp(leaky_relu(e_src[i]+e_dst[j])) by
exp(e_src[i]) (a per-i factor that cancels in the softmax) gives

  P'[j,i] = max( exp(-0.8*e_src[i]) * exp(0.2*e_dst[j]),  exp(e_dst[j]) )

i.e. ONE fused DVE tensor_scalar op per [128,1024] tile (mult + max against
two per-partition scalars) in bf16 -- no exps in the main loop at all.
Mask multiply runs as one [128,2048] bf16 tensor_tensor per HEAD PAIR
(adjacency duplicated along the free dim), all on DVE (gpsimd contends for
SBUF ports and slows DVE ~4x -- measured).  AV matmul in bf16 (1 cycle/row)
with a ones column appended to the lhsT so row 64 of the accumulator is the
softmax denominator.  Output staged per row-block and stored in 8 batched
DMAs.

Host-side prep (layout/dtype only): h.T, adj.T as bf16, W and W@A as bf16.
"""
import sys

sys.path.insert(0, "/opt/trn_rl_repo")

import numpy as np
import ml_dtypes

import concourse.bass as bass
import concourse.mybir as mybir
import concourse.tile as tile
from concourse.bass_utils import run_bass_kernel_spmd
from concourse.masks import make_identity

F32 = mybir.dt.float32
BF16 = mybir.dt.bfloat16
AF = mybir.ActivationFunctionType
ALU = mybir.AluOpType
BF16NP = ml_dtypes.bfloat16

N_CORES = 8
N = 1024
NB = 8          # row blocks of 128
FIN = 256
KT = 2          # FIN / 128
FO = 512        # heads * fo
H = 8
FOH = 64
ALPHA = 0.2

NSEL = 8        # all heads broadcast via PE selector matmuls

_MAX_SYNC_WAITS = 1


def _split_sync_waits(nc, max_waits=_MAX_SYNC_WAITS):
    """This walrus build rejects instructions carrying more than one sync
    wait; hoist extras onto NOPs inserted just before, on the same engine."""
    uid = 0
    for f in nc.m.functions:
        for bb in f.blocks:
            out = []
            for inst in bb.instructions:
                si = getattr(inst, "sync_info", None)
                if si is not None and si.on_wait and len(si.on_wait) > max_waits:
                    waits = list(si.on_wait)
                    keep = waits[-max_waits:]
                    extra = waits[:-max_waits]
                    si.on_wait.clear()
                    si.on_wait.extend(keep)
                    while extra:
                        chunk, extra = extra[:max_waits], extra[max_waits:]
                        nop = mybir.InstNoOp(
                            name=f"waitsplit-{uid}",
                            engine=inst.engine,
                            sync_info=mybir.SyncInfo(
                                on_wait=list(chunk), on_update=[]
                            ),
                            bass_nofuse=True,
                        )
                        uid += 1
                        out.append(nop)
                out.append(inst)
            bb.instructions[:] = out


def build_nc(split=True):
    nc = bass.Bass()
    hT_d = nc.declare_dram_parameter("hTb", [FIN, N], BF16, isOutput=False)
    adjT_d = nc.declare_dram_parameter("adjT2", [N, 2 * N], BF16, isOutput=False)
    w_d = nc.declare_dram_parameter("Wb", [FIN, FO], BF16, isOutput=False)
    wa_d = nc.declare_dram_parameter("WAb", [FIN, 2 * H], BF16, isOutput=False)
    out_d = nc.declare_dram_parameter("out", [N, FO], F32, isOutput=True)

    with tile.TileContext(nc) as tc:
        with (
            tc.tile_pool(name="const", bufs=1) as const,
            tc.tile_pool(name="persist", bufs=1) as persist,
            tc.tile_pool(name="tp8", bufs=5) as tpool,
            tc.tile_pool(name="epi", bufs=3) as epi,
            tc.tile_pool(name="psS", bufs=3, space="PSUM") as psS,
            tc.tile_pool(name="psAcc", bufs=1, space="PSUM") as psAcc,
        ):
            ident = const.tile([128, 128], F32, tag="ident")
            make_identity(nc, ident[:])

            wa = []
            for k in range(KT):
                t = const.tile([128, 2 * H], BF16, tag=f"WA{k}", name=f"WA{k}")
                nc.sync.dma_start(t[:], wa_d[k * 128:(k + 1) * 128, :])
                wa.append(t)
            hT = [persist.tile([128, N], BF16, tag=f"hT{k}", name=f"hT{k}")
                  for k in range(KT)]
            for k in range(KT):
                nc.sync.dma_start(hT[k][:], hT_d[k * 128:(k + 1) * 128, :])
            wk = []
            for k in range(KT):
                t = const.tile([128, FO], BF16, tag=f"W{k}", name=f"W{k}")
                nc.sync.dma_start(t[:], w_d[k * 128:(k + 1) * 128, :])
                wk.append(t)
            # adjacency duplicated along free dim (host-prepped): one TT
            # masks a head pair
            adjT2 = [persist.tile([128, 2 * N], BF16, tag=f"adjT{j}",
                                  name=f"adjT{j}")
                     for j in range(NB)]
            for jb in range(NB):
                nc.sync.dma_start(
                    adjT2[jb][:], adjT_d[jb * 128:(jb + 1) * 128, :]
                )

            # ---- E_T[16, i] = (WA.T @ hT): rows 0..7 e_src, 8..15 e_dst ----
            e_t = const.tile([16, N], F32, tag="eT")
            for c in range(2):
                ps = psS.tile([16, 512], F32, tag="ps")
                for k in range(KT):
                    nc.tensor.matmul(
                        ps[:], wa[k][:], hT[k][:, c * 512:(c + 1) * 512],
                        start=(k == 0), stop=(k == KT - 1),
                    )
                nc.vector.tensor_copy(e_t[:, c * 512:(c + 1) * 512], ps[:])

            # ---- G8[h, i] = exp(-(1-alpha) * e_src[h, i]) as bf16 ----
            g8 = const.tile([8, N], BF16, tag="g8")
            nc.scalar.activation(g8[:], e_t[0:8, :], AF.Exp, scale=-(1.0 - ALPHA))

            # ---- e_sb[jb][p, 16] = E_T[:, jb*128+p]; s0/s1 = per-j scalars ----
            e_sb = [persist.tile([128, 16], F32, tag=f"E{j}", name=f"E{j}")
                    for j in range(NB)]
            s0sb = [persist.tile([128, H], F32, tag=f"s0{j}", name=f"s0{j}")
                    for j in range(NB)]
            s1sb = [persist.tile([128, H], F32, tag=f"s1{j}", name=f"s1{j}")
                    for j in range(NB)]
            for jb in range(NB):
                tp = psS.tile([128, 512], F32, tag="ps")
                nc.tensor.transpose(
                    tp[:, 0:16], e_t[:, jb * 128:(jb + 1) * 128],
                    ident[0:16, 0:16],
                )
                nc.vector.tensor_copy(e_sb[jb][:], tp[:, 0:16])

            # ---- Gb broadcast over partitions via PE selector matmuls.
            # Emission order feeds pair 0 first: heads 0-1, then jb=0 s-cols,
            # then the rest -- PE and ACT are otherwise idle here. ----
            gbsel = [persist.tile([128, N], BF16, tag=f"gb{hh}", name=f"gb{hh}")
                     for hh in range(H)]
            sel = []
            for hh in range(H):
                t = const.tile([8, 128], BF16, tag=f"sel{hh}", name=f"sel{hh}")
                nc.gpsimd.memset(t[:], 0.0)
                nc.gpsimd.affine_select(
                    out=t[:], in_=t[:], pattern=[[0, 128]],
                    compare_op=ALU.not_equal, fill=1.0,
                    base=-hh, channel_multiplier=1,
                )
                sel.append(t)

            def bcast_head(hh):
                for c in range(2):
                    ps = psS.tile([128, 512], F32, tag="ps")
                    nc.tensor.matmul(
                        ps[:], sel[hh][:], g8[:, c * 512:(c + 1) * 512],
                        start=True, stop=True,
                    )
                    nc.scalar.copy(
                        gbsel[hh][:, c * 512:(c + 1) * 512], ps[:]
                    )

            def scols(jb):
                # s0 = exp(alpha * e_dst), s1 = exp(e_dst)
                nc.scalar.activation(
                    s0sb[jb][:], e_sb[jb][:, 8:16], AF.Exp, scale=ALPHA,
                )
                nc.scalar.activation(
                    s1sb[jb][:], e_sb[jb][:, 8:16], AF.Exp, scale=1.0,
                )

            for hh in range(2):
                bcast_head(hh)
            for jb in range(NB):
                scols(jb)

            def gb(hh):
                return gbsel[hh][:, :]

            # ---- wh_aug[jb][j, h, 0:64] = (h @ W) block bf16, [:, h, 64] = 1 ----
            wh_aug = [persist.tile([128, H, 65], BF16, tag=f"wha{j}",
                                   name=f"wha{j}")
                      for j in range(NB)]
            for jb in range(NB):
                ps = psS.tile([128, H, FOH], F32, tag="ps")
                for k in range(KT):
                    nc.tensor.matmul(
                        ps[:, :, :], hT[k][:, jb * 128:(jb + 1) * 128], wk[k][:],
                        start=(k == 0), stop=(k == KT - 1),
                    )
                nc.scalar.activation(
                    wh_aug[jb][:, :, 0:64], ps[:, :, :], AF.Copy,
                )
                nc.gpsimd.memset(wh_aug[jb][:, :, 64:65], 1.0)
            for hh in range(2, H):
                bcast_head(hh)

            # ---- output staging: osm_all[cb][p, h*64+f] ----
            osm_all = [persist.tile([128, FO], F32, tag=f"osm{c}",
                                    name=f"osm{c}")
                       for c in range(NB)]

            # ---- main attention loop, head pairs ----
            for hp in range(H // 2):
                h0, h1 = 2 * hp, 2 * hp + 1
                acc = {
                    (hh, c): psAcc.tile([65, 512], F32, tag=f"acc{hh % 2}{c}",
                                        name=f"acc{hh % 2}{c}")
                    for hh in (h0, h1) for c in range(2)
                }
                for jb in range(NB):
                    t2 = tpool.tile([128, 2 * N], BF16, tag="t2")
                    for q, hh in enumerate((h0, h1)):
                        nc.vector.tensor_scalar(
                            t2[:, q * N:(q + 1) * N], gb(hh),
                            s0sb[jb][:, hh:hh + 1], s1sb[jb][:, hh:hh + 1],
                            ALU.mult, ALU.max,
                        )
                    nc.vector.tensor_mul(t2[:], t2[:], adjT2[jb][:])
                    for q, hh in enumerate((h0, h1)):
                        for c in range(2):
                            nc.tensor.matmul(
                                acc[(hh, c)][:],
                                wh_aug[jb][:, hh, :],
                                t2[:, q * N + c * 512:q * N + (c + 1) * 512],
                                start=(jb == 0), stop=(jb == NB - 1),
                            )
                # epilogue: acc -> SBUF (ACT), transpose back, divide, stage
                for hh in (h0, h1):
                    acc_sb = epi.tile([65, N], F32, tag="accsb")
                    rec8 = epi.tile([128, 8], F32, tag="rec8")
                    for q in range(2):
                        nc.scalar.copy(
                            acc_sb[:, q * 512:(q + 1) * 512], acc[(hh, q)][:]
                        )
                        tp = psS.tile([128, 4 * 65], F32, tag="ps")
                        for r in range(4):
                            cb = q * 4 + r
                            nc.tensor.transpose(
                                tp[:, r * 65:r * 65 + 65],
                                acc_sb[:, cb * 128:(cb + 1) * 128],
                                ident[0:65, 0:65],
                            )
                        nc.vector.reciprocal(
                            rec8[:, q * 4:(q + 1) * 4], tp[:, 64::65]
                        )
                        for r in range(4):
                            cb = q * 4 + r
                            nc.scalar.activation(
                                osm_all[cb][:, hh * FOH:(hh + 1) * FOH],
                                tp[:, r * 65:r * 65 + 64], AF.Copy,
                                scale=rec8[:, cb:cb + 1],
                            )
                # flush this pair's 128-col quarter of the output
                for cb in range(NB):
                    nc.sync.dma_start(
                        out_d[cb * 128:(cb + 1) * 128,
                              hp * 128:(hp + 1) * 128],
                        osm_all[cb][:, hp * 128:(hp + 1) * 128],
                    )

    if split:
        _split_sync_waits(nc)
    return nc


_NC_CACHE = None


def _get_nc():
    global _NC_CACHE
    if _NC_CACHE is None:
        _NC_CACHE = build_nc()
    return _NC_CACHE


def _dup_adjT(adj_c):
    at = np.ascontiguousarray(adj_c.T).astype(BF16NP)
    return np.ascontiguousarray(np.concatenate([at, at], axis=1))


def _prep_in_maps(h, adj, W, a):
    h = np.ascontiguousarray(h, dtype=np.float32)
    adj = np.ascontiguousarray(adj, dtype=np.int32)
    W = np.ascontiguousarray(W, dtype=np.float32)
    a = np.ascontiguousarray(a, dtype=np.float32)
    amat = np.zeros((FO, 2 * H), dtype=np.float32)
    for hh in range(H):
        amat[hh * FOH:(hh + 1) * FOH, hh] = a[hh, :FOH]
        amat[hh * FOH:(hh + 1) * FOH, H + hh] = a[hh, FOH:]
    wamat = (W @ amat).astype(BF16NP)
    wb = W.astype(BF16NP)
    return [
        {
            "hTb": np.ascontiguousarray(h[c].T).astype(BF16NP),
            "adjT2": _dup_adjT(adj[c]),
            "Wb": wb,
            "WAb": wamat,
        }
        for c in range(N_CORES)
    ]


def run(h, adj, W, a, trace=False, **kw):
    nc = _get_nc()
    in_maps = _prep_in_maps(h, adj, W, a)
    res = run_bass_kernel_spmd(nc, in_maps, list(range(N_CORES)), trace=trace, **kw)
    out = np.stack([res.results[c]["out"] for c in range(N_CORES)], axis=0)
    return out.astype(np.float32), res


def kernel(h, adj, W, a):
    out, _ = run(h, adj, W, a)
    return out


# revision 10
# speedup vs baseline: 1.0411x; 1.0186x over previous
"""MultiHeadGAT layer on 8 trn2 NeuronCores, data-parallel over batch.

Per core (one batch element), with softmax-invariant rescaling: dividing the
unnormalized attention P[j,i] = exp(leaky_relu(e_src[i]+e_dst[j])) by
exp(e_src[i]) (a per-i factor that cancels in the softmax) gives

  P'[j,i] = max( exp(-0.8*e_src[i]) * exp(0.2*e_dst[j]),  exp(e_dst[j]) )

i.e. ONE fused DVE tensor_scalar op per [128,1024] tile (mult + max against
two per-partition scalars) in bf16 -- no exps in the main loop at all.
Mask multiply runs as one [128,2048] bf16 tensor_tensor per HEAD PAIR
(adjacency duplicated along the free dim), all on DVE (gpsimd contends for
SBUF ports and slows DVE ~4x -- measured).  AV matmul in bf16 (1 cycle/row)
with a ones column appended to the lhsT so row 64 of the accumulator is the
softmax denominator.  Output staged per row-block and stored in 8 batched
DMAs.

Host-side prep (layout/dtype only): h.T, adj.T as bf16, W and W@A as bf16.
"""
import sys

sys.path.insert(0, "/opt/trn_rl_repo")

import numpy as np
import ml_dtypes

import concourse.bass as bass
import concourse.mybir as mybir
import concourse.tile as tile
from concourse.bass_utils import run_bass_kernel_spmd
from concourse.masks import make_identity

F32 = mybir.dt.float32
BF16 = mybir.dt.bfloat16
AF = mybir.ActivationFunctionType
ALU = mybir.AluOpType
BF16NP = ml_dtypes.bfloat16

N_CORES = 8
N = 1024
NB = 8          # row blocks of 128
FIN = 256
KT = 2          # FIN / 128
FO = 512        # heads * fo
H = 8
FOH = 64
ALPHA = 0.2

NSEL = 4        # heads 0-3 via PE selector; 4-7 via parallel DMA doubling

_MAX_SYNC_WAITS = 1


def _split_sync_waits(nc, max_waits=_MAX_SYNC_WAITS):
    """This walrus build rejects instructions carrying more than one sync
    wait; hoist extras onto NOPs inserted just before, on the same engine."""
    uid = 0
    for f in nc.m.functions:
        for bb in f.blocks:
            out = []
            for inst in bb.instructions:
                si = getattr(inst, "sync_info", None)
                if si is not None and si.on_wait and len(si.on_wait) > max_waits:
                    waits = list(si.on_wait)
                    keep = waits[-max_waits:]
                    extra = waits[:-max_waits]
                    si.on_wait.clear()
                    si.on_wait.extend(keep)
                    while extra:
                        chunk, extra = extra[:max_waits], extra[max_waits:]
                        nop = mybir.InstNoOp(
                            name=f"waitsplit-{uid}",
                            engine=inst.engine,
                            sync_info=mybir.SyncInfo(
                                on_wait=list(chunk), on_update=[]
                            ),
                            bass_nofuse=True,
                        )
                        uid += 1
                        out.append(nop)
                out.append(inst)
            bb.instructions[:] = out


def build_nc(split=True):
    nc = bass.Bass()
    hT_d = nc.declare_dram_parameter("hTb", [FIN, N], BF16, isOutput=False)
    adjT_d = nc.declare_dram_parameter("adjT2", [N, 2 * N], BF16, isOutput=False)
    w_d = nc.declare_dram_parameter("Wb", [FIN, FO], BF16, isOutput=False)
    wa_d = nc.declare_dram_parameter("WAb", [FIN, 2 * H], BF16, isOutput=False)
    out_d = nc.declare_dram_parameter("out", [N, FO], F32, isOutput=True)

    with tile.TileContext(nc) as tc:
        with (
            tc.tile_pool(name="const", bufs=1) as const,
            tc.tile_pool(name="persist", bufs=1) as persist,
            tc.tile_pool(name="tp8", bufs=5) as tpool,
            tc.tile_pool(name="epi", bufs=3) as epi,
            tc.tile_pool(name="psS", bufs=3, space="PSUM") as psS,
            tc.tile_pool(name="psAcc", bufs=1, space="PSUM") as psAcc,
        ):
            ident = const.tile([128, 128], F32, tag="ident")
            make_identity(nc, ident[:])

            wa = []
            for k in range(KT):
                t = const.tile([128, 2 * H], BF16, tag=f"WA{k}", name=f"WA{k}")
                nc.sync.dma_start(t[:], wa_d[k * 128:(k + 1) * 128, :])
                wa.append(t)
            hT = [persist.tile([128, N], BF16, tag=f"hT{k}", name=f"hT{k}")
                  for k in range(KT)]
            for k in range(KT):
                nc.sync.dma_start(hT[k][:], hT_d[k * 128:(k + 1) * 128, :])
            wk = []
            for k in range(KT):
                t = const.tile([128, FO], BF16, tag=f"W{k}", name=f"W{k}")
                nc.sync.dma_start(t[:], w_d[k * 128:(k + 1) * 128, :])
                wk.append(t)
            # adjacency duplicated along free dim (host-prepped): one TT
            # masks a head pair
            adjT2 = [persist.tile([128, 2 * N], BF16, tag=f"adjT{j}",
                                  name=f"adjT{j}")
                     for j in range(NB)]
            for jb in range(NB):
                nc.sync.dma_start(
                    adjT2[jb][:], adjT_d[jb * 128:(jb + 1) * 128, :]
                )

            # ---- E_T[16, i] = (WA.T @ hT): rows 0..7 e_src, 8..15 e_dst ----
            e_t = const.tile([16, N], F32, tag="eT")
            for c in range(2):
                ps = psS.tile([16, 512], F32, tag="ps")
                for k in range(KT):
                    nc.tensor.matmul(
                        ps[:], wa[k][:], hT[k][:, c * 512:(c + 1) * 512],
                        start=(k == 0), stop=(k == KT - 1),
                    )
                nc.vector.tensor_copy(e_t[:, c * 512:(c + 1) * 512], ps[:])

            # ---- G8[h, i] = exp(-(1-alpha) * e_src[h, i]) as bf16 ----
            g8 = const.tile([8, N], BF16, tag="g8")
            nc.scalar.activation(g8[:], e_t[0:8, :], AF.Exp, scale=-(1.0 - ALPHA))

            # ---- e_sb[jb][p, 16] = E_T[:, jb*128+p]; s0/s1 = per-j scalars ----
            e_sb = [persist.tile([128, 16], F32, tag=f"E{j}", name=f"E{j}")
                    for j in range(NB)]
            s0sb = [persist.tile([128, H], F32, tag=f"s0{j}", name=f"s0{j}")
                    for j in range(NB)]
            s1sb = [persist.tile([128, H], F32, tag=f"s1{j}", name=f"s1{j}")
                    for j in range(NB)]
            for jb in range(NB):
                tp = psS.tile([128, 512], F32, tag="ps")
                nc.tensor.transpose(
                    tp[:, 0:16], e_t[:, jb * 128:(jb + 1) * 128],
                    ident[0:16, 0:16],
                )
                nc.vector.tensor_copy(e_sb[jb][:], tp[:, 0:16])

            # ---- Gb broadcast over partitions via PE selector matmuls.
            # Emission order feeds pair 0 first: heads 0-1, then jb=0 s-cols,
            # then the rest -- PE and ACT are otherwise idle here. ----
            gbsel = [persist.tile([128, N], BF16, tag=f"gb{hh}", name=f"gb{hh}")
                     for hh in range(H)]
            sel = []
            for hh in range(NSEL):
                t = const.tile([8, 128], BF16, tag=f"sel{hh}", name=f"sel{hh}")
                nc.gpsimd.memset(t[:], 0.0)
                nc.gpsimd.affine_select(
                    out=t[:], in_=t[:], pattern=[[0, 128]],
                    compare_op=ALU.not_equal, fill=1.0,
                    base=-hh, channel_multiplier=1,
                )
                sel.append(t)

            def bcast_head(hh):
                for c in range(2):
                    ps = psS.tile([128, 512], F32, tag="ps")
                    nc.tensor.matmul(
                        ps[:], sel[hh][:], g8[:, c * 512:(c + 1) * 512],
                        start=True, stop=True,
                    )
                    nc.scalar.copy(
                        gbsel[hh][:, c * 512:(c + 1) * 512], ps[:]
                    )

            def scols(jb):
                # s0 = exp(alpha * e_dst), s1 = exp(e_dst)
                nc.scalar.activation(
                    s0sb[jb][:], e_sb[jb][:, 8:16], AF.Exp, scale=ALPHA,
                )
                nc.scalar.activation(
                    s1sb[jb][:], e_sb[jb][:, 8:16], AF.Exp, scale=1.0,
                )

            for hh in range(2):
                bcast_head(hh)
            for hh in range(NSEL, H):
                t = gbsel[hh]
                nc.sync.dma_start(t[0:1, :], g8[hh:hh + 1, :])
                p = 1
                while p < 128:
                    nc.sync.dma_start(t[p:2 * p, :], t[0:p, :])
                    p *= 2
            for jb in range(NB):
                scols(jb)

            def gb(hh):
                return gbsel[hh][:, :]

            # ---- wh_aug[jb][j, h, 0:64] = (h @ W) block bf16, [:, h, 64] = 1 ----
            wh_aug = [persist.tile([128, H, 65], BF16, tag=f"wha{j}",
                                   name=f"wha{j}")
                      for j in range(NB)]
            for jb in range(NB):
                ps = psS.tile([128, H, FOH], F32, tag="ps")
                for k in range(KT):
                    nc.tensor.matmul(
                        ps[:, :, :], hT[k][:, jb * 128:(jb + 1) * 128], wk[k][:],
                        start=(k == 0), stop=(k == KT - 1),
                    )
                nc.scalar.activation(
                    wh_aug[jb][:, :, 0:64], ps[:, :, :], AF.Copy,
                )
                nc.gpsimd.memset(wh_aug[jb][:, :, 64:65], 1.0)
            for hh in range(2, NSEL):
                bcast_head(hh)

            # ---- output staging: osm_all[cb][p, h*64+f] ----
            osm_all = [persist.tile([128, FO], F32, tag=f"osm{c}",
                                    name=f"osm{c}")
                       for c in range(NB)]

            # ---- main attention loop, head pairs ----
            for hp in range(H // 2):
                h0, h1 = 2 * hp, 2 * hp + 1
                acc = {
                    (hh, c): psAcc.tile([65, 512], F32, tag=f"acc{hh % 2}{c}",
                                        name=f"acc{hh % 2}{c}")
                    for hh in (h0, h1) for c in range(2)
                }
                for jb in range(NB):
                    t2 = tpool.tile([128, 2 * N], BF16, tag="t2")
                    for q, hh in enumerate((h0, h1)):
                        nc.vector.tensor_scalar(
                            t2[:, q * N:(q + 1) * N], gb(hh),
                            s0sb[jb][:, hh:hh + 1], s1sb[jb][:, hh:hh + 1],
                            ALU.mult, ALU.max,
                        )
                    nc.vector.tensor_mul(t2[:], t2[:], adjT2[jb][:])
                    for q, hh in enumerate((h0, h1)):
                        for c in range(2):
                            nc.tensor.matmul(
                                acc[(hh, c)][:],
                                wh_aug[jb][:, hh, :],
                                t2[:, q * N + c * 512:q * N + (c + 1) * 512],
                                start=(jb == 0), stop=(jb == NB - 1),
                            )
                # epilogue: acc -> SBUF (ACT), transpose back, divide, stage
                for hh in (h0, h1):
                    acc_sb = epi.tile([65, N], F32, tag="accsb")
                    rec8 = epi.tile([128, 8], F32, tag="rec8")
                    for q in range(2):
                        nc.scalar.copy(
                            acc_sb[:, q * 512:(q + 1) * 512], acc[(hh, q)][:]
                        )
                        tp = psS.tile([128, 4 * 65], F32, tag="ps")
                        for r in range(4):
                            cb = q * 4 + r
                            nc.tensor.transpose(
                                tp[:, r * 65:r * 65 + 65],
                                acc_sb[:, cb * 128:(cb + 1) * 128],
                                ident[0:65, 0:65],
                            )
                        nc.vector.reciprocal(
                            rec8[:, q * 4:(q + 1) * 4], tp[:, 64::65]
                        )
                        for r in range(4):
                            cb = q * 4 + r
                            nc.scalar.activation(
                                osm_all[cb][:, hh * FOH:(hh + 1) * FOH],
                                tp[:, r * 65:r * 65 + 64], AF.Copy,
                                scale=rec8[:, cb:cb + 1],
                            )
                            # flush this pair's 128-col quarter per block as
                            # soon as the second head's slice lands
                            if hh == h1:
                                nc.sync.dma_start(
                                    out_d[cb * 128:(cb + 1) * 128,
                                          hp * 128:(hp + 1) * 128],
                                    osm_all[cb][:, hp * 128:(hp + 1) * 128],
                                )

    if split:
        _split_sync_waits(nc)
    return nc


_NC_CACHE = None


def _get_nc():
    global _NC_CACHE
    if _NC_CACHE is None:
        _NC_CACHE = build_nc()
    return _NC_CACHE


def _dup_adjT(adj_c):
    at = np.ascontiguousarray(adj_c.T).astype(BF16NP)
    return np.ascontiguousarray(np.concatenate([at, at], axis=1))


def _prep_in_maps(h, adj, W, a):
    h = np.ascontiguousarray(h, dtype=np.float32)
    adj = np.ascontiguousarray(adj, dtype=np.int32)
    W = np.ascontiguousarray(W, dtype=np.float32)
    a = np.ascontiguousarray(a, dtype=np.float32)
    amat = np.zeros((FO, 2 * H), dtype=np.float32)
    for hh in range(H):
        amat[hh * FOH:(hh + 1) * FOH, hh] = a[hh, :FOH]
        amat[hh * FOH:(hh + 1) * FOH, H + hh] = a[hh, FOH:]
    wamat = (W @ amat).astype(BF16NP)
    wb = W.astype(BF16NP)
    return [
        {
            "hTb": np.ascontiguousarray(h[c].T).astype(BF16NP),
            "adjT2": _dup_adjT(adj[c]),
            "Wb": wb,
            "WAb": wamat,
        }
        for c in range(N_CORES)
    ]


def run(h, adj, W, a, trace=False, **kw):
    nc = _get_nc()
    in_maps = _prep_in_maps(h, adj, W, a)
    res = run_bass_kernel_spmd(nc, in_maps, list(range(N_CORES)), trace=trace, **kw)
    out = np.stack([res.results[c]["out"] for c in range(N_CORES)], axis=0)
    return out.astype(np.float32), res


def kernel(h, adj, W, a):
    out, _ = run(h, adj, W, a)
    return out


# revision 11
# speedup vs baseline: 1.2112x; 1.1634x over previous
"""MultiHeadGAT layer on 8 trn2 NeuronCores, data-parallel over batch.

Per core (one batch element), with softmax-invariant rescaling: dividing the
unnormalized attention P[j,i] = exp(leaky_relu(e_src[i]+e_dst[j])) by
exp(e_src[i]) (a per-i factor that cancels in the softmax) gives

  P'[j,i] = max( exp(-0.8*e_src[i]) * exp(0.2*e_dst[j]),  exp(e_dst[j]) )

i.e. ONE fused DVE tensor_scalar op per [128,1024] tile (mult + max against
two per-partition scalars) in bf16 -- no exps in the main loop at all.
Mask multiply runs as one [128,2048] bf16 tensor_tensor per HEAD PAIR
(adjacency duplicated along the free dim), all on DVE (gpsimd contends for
SBUF ports and slows DVE ~4x -- measured).  AV matmul in bf16 (1 cycle/row)
with a ones column appended to the lhsT so row 64 of the accumulator is the
softmax denominator.  Output staged per row-block and stored in 8 batched
DMAs.

Host-side prep (layout/dtype only): h.T, adj.T as bf16, W and W@A as bf16.
"""
import sys

sys.path.insert(0, "/opt/trn_rl_repo")

import numpy as np
import ml_dtypes

import concourse.bass as bass
import concourse.mybir as mybir
import concourse.tile as tile
from concourse.bass_utils import run_bass_kernel_spmd
from concourse.masks import make_identity

F32 = mybir.dt.float32
BF16 = mybir.dt.bfloat16
AF = mybir.ActivationFunctionType
ALU = mybir.AluOpType
BF16NP = ml_dtypes.bfloat16

N_CORES = 8
N = 1024
NB = 8          # row blocks of 128
FIN = 256
KT = 2          # FIN / 128
FO = 512        # heads * fo
H = 8
FOH = 64
ALPHA = 0.2

NSEL = 4        # heads 0-3 via PE selector; 4-7 via parallel DMA doubling

_MAX_SYNC_WAITS = 1


def _split_sync_waits(nc, max_waits=_MAX_SYNC_WAITS):
    """This walrus build rejects instructions carrying more than one sync
    wait; hoist extras onto NOPs inserted just before, on the same engine."""
    uid = 0
    for f in nc.m.functions:
        for bb in f.blocks:
            out = []
            for inst in bb.instructions:
                si = getattr(inst, "sync_info", None)
                if si is not None and si.on_wait and len(si.on_wait) > max_waits:
                    waits = list(si.on_wait)
                    keep = waits[-max_waits:]
                    extra = waits[:-max_waits]
                    si.on_wait.clear()
                    si.on_wait.extend(keep)
                    while extra:
                        chunk, extra = extra[:max_waits], extra[max_waits:]
                        nop = mybir.InstNoOp(
                            name=f"waitsplit-{uid}",
                            engine=inst.engine,
                            sync_info=mybir.SyncInfo(
                                on_wait=list(chunk), on_update=[]
                            ),
                            bass_nofuse=True,
                        )
                        uid += 1
                        out.append(nop)
                out.append(inst)
            bb.instructions[:] = out


def build_nc(split=True):
    nc = bass.Bass()
    hT_d = nc.declare_dram_parameter("hTb", [FIN, N], BF16, isOutput=False)
    adjT_d = nc.declare_dram_parameter("adjT2", [N, 2 * N], BF16, isOutput=False)
    w_d = nc.declare_dram_parameter("Wb", [FIN, FO], BF16, isOutput=False)
    wa_d = nc.declare_dram_parameter("WAb", [FIN, 2 * H], BF16, isOutput=False)
    out_d = nc.declare_dram_parameter("out", [N, FO], F32, isOutput=True)

    with tile.TileContext(nc) as tc:
        with (
            tc.tile_pool(name="const", bufs=1) as const,
            tc.tile_pool(name="persist", bufs=1) as persist,
            tc.tile_pool(name="tp8", bufs=5) as tpool,
            tc.tile_pool(name="epi", bufs=3) as epi,
            tc.tile_pool(name="psS", bufs=3, space="PSUM") as psS,
            tc.tile_pool(name="psAcc", bufs=1, space="PSUM") as psAcc,
        ):
            ident = const.tile([128, 128], F32, tag="ident")
            make_identity(nc, ident[:])

            wa = []
            for k in range(KT):
                t = const.tile([128, 2 * H], BF16, tag=f"WA{k}", name=f"WA{k}")
                nc.sync.dma_start(t[:], wa_d[k * 128:(k + 1) * 128, :])
                wa.append(t)
            hT = [persist.tile([128, N], BF16, tag=f"hT{k}", name=f"hT{k}")
                  for k in range(KT)]
            for k in range(KT):
                nc.sync.dma_start(hT[k][:], hT_d[k * 128:(k + 1) * 128, :])
            wk = []
            for k in range(KT):
                t = const.tile([128, FO], BF16, tag=f"W{k}", name=f"W{k}")
                nc.sync.dma_start(t[:], w_d[k * 128:(k + 1) * 128, :])
                wk.append(t)
            # adjacency duplicated along free dim (host-prepped): one TT
            # masks a head pair
            adjT2 = [persist.tile([128, 2 * N], BF16, tag=f"adjT{j}",
                                  name=f"adjT{j}")
                     for j in range(NB)]
            for jb in range(NB):
                nc.sync.dma_start(
                    adjT2[jb][:], adjT_d[jb * 128:(jb + 1) * 128, :]
                )

            # ---- E_T[16, i] = (WA.T @ hT): rows 0..7 e_src, 8..15 e_dst ----
            e_t = const.tile([16, N], F32, tag="eT")
            for c in range(2):
                ps = psS.tile([16, 512], F32, tag="ps")
                for k in range(KT):
                    nc.tensor.matmul(
                        ps[:], wa[k][:], hT[k][:, c * 512:(c + 1) * 512],
                        start=(k == 0), stop=(k == KT - 1),
                    )
                nc.vector.tensor_copy(e_t[:, c * 512:(c + 1) * 512], ps[:])

            # ---- G8[h, i] = exp(-(1-alpha) * e_src[h, i]) as bf16 ----
            g8 = const.tile([8, N], BF16, tag="g8")
            nc.scalar.activation(g8[:], e_t[0:8, :], AF.Exp, scale=-(1.0 - ALPHA))

            # ---- e_sb[jb][p, 16] = E_T[:, jb*128+p]; s0/s1 = per-j scalars ----
            e_sb = [persist.tile([128, 16], F32, tag=f"E{j}", name=f"E{j}")
                    for j in range(NB)]
            s0sb = [persist.tile([128, H], F32, tag=f"s0{j}", name=f"s0{j}")
                    for j in range(NB)]
            s1sb = [persist.tile([128, H], F32, tag=f"s1{j}", name=f"s1{j}")
                    for j in range(NB)]
            def esb(jb):
                tp = psS.tile([128, 512], F32, tag="ps")
                nc.tensor.transpose(
                    tp[:, 0:16], e_t[:, jb * 128:(jb + 1) * 128],
                    ident[0:16, 0:16],
                )
                nc.vector.tensor_copy(e_sb[jb][:], tp[:, 0:16])

            esb(0)

            # ---- Gb broadcast over partitions via PE selector matmuls.
            # Emission order feeds pair 0 first: heads 0-1, then jb=0 s-cols,
            # then the rest -- PE and ACT are otherwise idle here. ----
            gbsel = [persist.tile([128, N], BF16, tag=f"gb{hh}", name=f"gb{hh}")
                     for hh in range(H)]
            sel = []
            for hh in range(NSEL):
                t = const.tile([8, 128], BF16, tag=f"sel{hh}", name=f"sel{hh}")
                nc.gpsimd.memset(t[:], 0.0)
                nc.gpsimd.affine_select(
                    out=t[:], in_=t[:], pattern=[[0, 128]],
                    compare_op=ALU.not_equal, fill=1.0,
                    base=-hh, channel_multiplier=1,
                )
                sel.append(t)

            def bcast_head(hh):
                for c in range(2):
                    ps = psS.tile([128, 512], F32, tag="ps")
                    nc.tensor.matmul(
                        ps[:], sel[hh][:], g8[:, c * 512:(c + 1) * 512],
                        start=True, stop=True,
                    )
                    nc.scalar.copy(
                        gbsel[hh][:, c * 512:(c + 1) * 512], ps[:]
                    )

            def scols(jb):
                # s0 = exp(alpha * e_dst), s1 = exp(e_dst)
                nc.scalar.activation(
                    s0sb[jb][:], e_sb[jb][:, 8:16], AF.Exp, scale=ALPHA,
                )
                nc.scalar.activation(
                    s1sb[jb][:], e_sb[jb][:, 8:16], AF.Exp, scale=1.0,
                )

            bcast_head(0)
            scols(0)
            bcast_head(1)
            for jb in range(1, NB):
                esb(jb)
            for hh in range(NSEL, H):
                t = gbsel[hh]
                nc.sync.dma_start(t[0:1, :], g8[hh:hh + 1, :])
                p = 1
                while p < 128:
                    nc.sync.dma_start(t[p:2 * p, :], t[0:p, :])
                    p *= 2
            for jb in range(1, NB):
                scols(jb)

            def gb(hh):
                return gbsel[hh][:, :]

            # ---- wh_aug[jb][j, h, 0:64] = (h @ W) block bf16, [:, h, 64] = 1 ----
            wh_aug = [persist.tile([128, H, 65], BF16, tag=f"wha{j}",
                                   name=f"wha{j}")
                      for j in range(NB)]
            for jb in range(NB):
                ps = psS.tile([128, H, FOH], F32, tag="ps")
                for k in range(KT):
                    nc.tensor.matmul(
                        ps[:, :, :], hT[k][:, jb * 128:(jb + 1) * 128], wk[k][:],
                        start=(k == 0), stop=(k == KT - 1),
                    )
                nc.scalar.activation(
                    wh_aug[jb][:, :, 0:64], ps[:, :, :], AF.Copy,
                )
                nc.gpsimd.memset(wh_aug[jb][:, :, 64:65], 1.0)
            for hh in range(2, NSEL):
                bcast_head(hh)

            # ---- output staging: osm_all[cb][p, h*64+f] ----
            osm_all = [persist.tile([128, FO], F32, tag=f"osm{c}",
                                    name=f"osm{c}")
                       for c in range(NB)]

            # ---- main attention loop, head pairs ----
            for hp in range(H // 2):
                h0, h1 = 2 * hp, 2 * hp + 1
                acc = {
                    (hh, c): psAcc.tile([65, 512], F32, tag=f"acc{hh % 2}{c}",
                                        name=f"acc{hh % 2}{c}")
                    for hh in (h0, h1) for c in range(2)
                }
                for jb in range(NB):
                    t2 = tpool.tile([128, 2 * N], BF16, tag="t2")
                    for q, hh in enumerate((h0, h1)):
                        nc.vector.tensor_scalar(
                            t2[:, q * N:(q + 1) * N], gb(hh),
                            s0sb[jb][:, hh:hh + 1], s1sb[jb][:, hh:hh + 1],
                            ALU.mult, ALU.max,
                        )
                    nc.vector.tensor_mul(t2[:], t2[:], adjT2[jb][:])
                    for q, hh in enumerate((h0, h1)):
                        for c in range(2):
                            nc.tensor.matmul(
                                acc[(hh, c)][:],
                                wh_aug[jb][:, hh, :],
                                t2[:, q * N + c * 512:q * N + (c + 1) * 512],
                                start=(jb == 0), stop=(jb == NB - 1),
                            )
                # epilogue: acc -> SBUF (ACT), transpose back, divide, stage
                for hh in (h0, h1):
                    acc_sb = epi.tile([65, N], F32, tag="accsb")
                    rec8 = epi.tile([128, 8], F32, tag="rec8")
                    for q in range(2):
                        nc.scalar.copy(
                            acc_sb[:, q * 512:(q + 1) * 512], acc[(hh, q)][:]
                        )
                        tp = psS.tile([128, 4 * 65], F32, tag="ps")
                        for r in range(4):
                            cb = q * 4 + r
                            nc.tensor.transpose(
                                tp[:, r * 65:r * 65 + 65],
                                acc_sb[:, cb * 128:(cb + 1) * 128],
                                ident[0:65, 0:65],
                            )
                        nc.vector.reciprocal(
                            rec8[:, q * 4:(q + 1) * 4], tp[:, 64::65]
                        )
                        for r in range(4):
                            cb = q * 4 + r
                            if hp == H // 2 - 1:
                                nc.vector.tensor_scalar(
                                    osm_all[cb][:, hh * FOH:(hh + 1) * FOH],
                                    tp[:, r * 65:r * 65 + 64],
                                    rec8[:, cb:cb + 1], None, ALU.mult,
                                )
                            else:
                                nc.scalar.activation(
                                    osm_all[cb][:, hh * FOH:(hh + 1) * FOH],
                                    tp[:, r * 65:r * 65 + 64], AF.Copy,
                                    scale=rec8[:, cb:cb + 1],
                                )
                            # flush this pair's 128-col quarter per block as
                            # soon as the second head's slice lands
                            if hh == h1:
                                nc.sync.dma_start(
                                    out_d[cb * 128:(cb + 1) * 128,
                                          hp * 128:(hp + 1) * 128],
                                    osm_all[cb][:, hp * 128:(hp + 1) * 128],
                                )

    if split:
        _split_sync_waits(nc)
    return nc


_NC_CACHE = None


def _get_nc():
    global _NC_CACHE
    if _NC_CACHE is None:
        _NC_CACHE = build_nc()
    return _NC_CACHE


def _dup_adjT(adj_c):
    at = np.ascontiguousarray(adj_c.T).astype(BF16NP)
    return np.ascontiguousarray(np.concatenate([at, at], axis=1))


def _prep_in_maps(h, adj, W, a):
    h = np.ascontiguousarray(h, dtype=np.float32)
    adj = np.ascontiguousarray(adj, dtype=np.int32)
    W = np.ascontiguousarray(W, dtype=np.float32)
    a = np.ascontiguousarray(a, dtype=np.float32)
    amat = np.zeros((FO, 2 * H), dtype=np.float32)
    for hh in range(H):
        amat[hh * FOH:(hh + 1) * FOH, hh] = a[hh, :FOH]
        amat[hh * FOH:(hh + 1) * FOH, H + hh] = a[hh, FOH:]
    wamat = (W @ amat).astype(BF16NP)
    wb = W.astype(BF16NP)
    return [
        {
            "hTb": np.ascontiguousarray(h[c].T).astype(BF16NP),
            "adjT2": _dup_adjT(adj[c]),
            "Wb": wb,
            "WAb": wamat,
        }
        for c in range(N_CORES)
    ]


def run(h, adj, W, a, trace=False, **kw):
    nc = _get_nc()
    in_maps = _prep_in_maps(h, adj, W, a)
    res = run_bass_kernel_spmd(nc, in_maps, list(range(N_CORES)), trace=trace, **kw)
    out = np.stack([res.results[c]["out"] for c in range(N_CORES)], axis=0)
    return out.astype(np.float32), res


def kernel(h, adj, W, a):
    out, _ = run(h, adj, W, a)
    return out
